# revision 4
# baseline (speedup 1.0000x reference)
"""DiGCN inception-block (3 layers, 2 adjacencies) on 8 TRN2 NeuronCores.

Strategy (dst-sharded graph parallelism):
  - Nodes are partitioned across the 8 cores (12544 rows each, node space
    padded to 100352). Each core owns the output rows for its node shard.
  - Per layer: x_{k+1} = x_k @ W0 + (A1 @ x_k) @ W1 + (A2 @ x_k) @ W2 + b
    (using A @ (x W) == (A x) W, so the sparse ops run on raw x).
  - Sparse op A @ x: edges are sorted by destination block (128 dst rows)
    on the host, grouped into chunks of 128 edges. For each chunk the
    source rows are fetched with dma_gather (bf16, 256B rows) from the
    replicated x table in HBM, and a one-hot matrix
    onehot[e, d] = attr[e] * (dstrel[e] == d) is built on the vector
    engine with a single fused tensor_scalar(is_equal, mult). The
    TensorEngine then accumulates psum[feat, dst] += G_chunk.T @ onehot
    over all chunks of the block (PSUM accumulation = segment sum).
  - dma_gather indices are int16, so the x table is addressed in 4 ranges
    of 25088 rows; each (block, range) group is padded to a uniform chunk
    count so the SPMD program is identical on every core.
  - Dense part: per 128-node block, out_psum[node, feat] accumulates
    s1T/s2T/xT slices (feat-major lhsT) against the 128x128 weights.
  - Between layers the bf16 node shards are AllGathered into the next
    x table (internal Shared DRAM); layer 3 writes f32 shards directly.
"""

import sys

sys.path.insert(0, "/opt/trn_rl_repo")

import numpy as np
import ml_dtypes

from concourse import bass, mybir, bacc
import concourse.tile as tile
from concourse.bass_utils import run_bass_kernel_spmd

BF16 = ml_dtypes.bfloat16

NCORES = 8
F = 128  # feature dim (both in and out)
N = 100000  # real node count
NPAD = 100352  # 8 * 12544, multiple of 8*128
R = 4  # src ranges (int16 gather index limit)


def _prep_adjacency(src, dst, attr, NPAD):
    """Pack one adjacency into the uniform per-core grid.

    Returns (CPR-independent) per-core intermediate lists; final arrays are
    built once a global CPR is chosen.
    """
    NL = NPAD // NCORES
    B = NL // 128
    SR = NPAD // R
    per_core = []
    core = dst // NL
    for r in range(NCORES):
        m = core == r
        s = src[m].astype(np.int64)
        d = (dst[m] - r * NL).astype(np.int64)
        a = attr[m].astype(np.float32)
        b = d >> 7
        drel = (d & 127).astype(np.float32)
        q = s // SR
        srel = (s - q * SR).astype(np.int16)
        key = (b * R + q).astype(np.int64)
        order = np.argsort(key, kind="stable")
        key_s = key[order]
        counts = np.bincount(key_s, minlength=B * R)
        starts = np.concatenate([[0], np.cumsum(counts)[:-1]])
        pos = np.arange(len(key_s)) - starts[key_s]
        per_core.append((key_s, pos, srel[order], drel[order], a[order], counts))
    max_count = max(int(pc[5].max()) for pc in per_core) if len(src) else 0
    return per_core, max_count


def _finalize_adjacency(per_core, CPR, NPAD):
    NL = NPAD // NCORES
    B = NL // 128
    CB = R * CPR
    cap = CPR * 128
    idx_arrs, drel_arrs, attr_arrs = [], [], []
    for key_s, pos, srel, drel, a, counts in per_core:
        grid_src = np.zeros((B, R, cap), np.int16)
        grid_drel = np.zeros((B, R, cap), np.float32)
        grid_attr = np.zeros((B, R, cap), np.float32)
        bq_b = key_s // R
        bq_q = key_s % R
        grid_src[bq_b, bq_q, pos] = srel
        grid_drel[bq_b, bq_q, pos] = drel
        grid_attr[bq_b, bq_q, pos] = a
        # idx input: ranges-major, block-major inside: [R, B, cap] tokens.
        tokens = grid_src.transpose(1, 0, 2).reshape(-1)  # [R*B*cap]
        wrapped = np.tile(tokens.reshape(-1, 16).T, (8, 1))  # [128, R*B*cap/16]
        idx_arrs.append(np.ascontiguousarray(wrapped))
        # dstrel/attr: [128, B*CB] with col = b*CB + q*CPR + s, row = p
        dr = grid_drel.reshape(B, R, CPR, 128).transpose(3, 0, 1, 2).reshape(128, B * CB)
        at = grid_attr.reshape(B, R, CPR, 128).transpose(3, 0, 1, 2).reshape(128, B * CB)
        drel_arrs.append(np.ascontiguousarray(dr))
        attr_arrs.append(np.ascontiguousarray(at))
    return idx_arrs, drel_arrs, attr_arrs


def _build_kernel(NPAD, CPR):
    NL = NPAD // NCORES
    B = NL // 128
    CB = R * CPR
    SR = NPAD // R
    IDXW = R * B * CPR * 8  # idx free dim (int16 cols)
    # dma_gather is limited to 1024 indices per call; split each
    # (block, range) group into slabs of <=8 chunk-slots.
    SLAB = 8
    n_slabs = (CPR + SLAB - 1) // SLAB

    nc = bacc.Bacc("TRN2", target_bir_lowering=False, debug=False, num_devices=NCORES)
    dt = mybir.dt
    x_table = nc.declare_dram_parameter("input0", [NPAD, F], dt.bfloat16, isOutput=False)
    xT0_in = nc.declare_dram_parameter("input1", [128, NL], dt.bfloat16, isOutput=False)
    idx_in = [
        nc.declare_dram_parameter(f"input{2 + i}", [128, IDXW], dt.int16, isOutput=False)
        for i in range(2)
    ]
    drel_in = [
        nc.declare_dram_parameter(f"input{4 + i}", [128, B * CB], dt.float32, isOutput=False)
        for i in range(2)
    ]
    attr_in = [
        nc.declare_dram_parameter(f"input{6 + i}", [128, B * CB], dt.float32, isOutput=False)
        for i in range(2)
    ]
    w_in = nc.declare_dram_parameter("input8", [9 * 128, F], dt.bfloat16, isOutput=False)
    bias_in = nc.declare_dram_parameter("input9", [128, 3 * F], dt.float32, isOutput=False)
    iota_in = nc.declare_dram_parameter("input10", [128, 128], dt.bfloat16, isOutput=False)
    out_p = nc.declare_dram_parameter("output0", [NL, F], dt.float32, isOutput=True)

    table1 = nc.dram_tensor("table1", [NPAD, F], dt.bfloat16, addr_space="Shared")
    table2 = nc.dram_tensor("table2", [NPAD, F], dt.bfloat16, addr_space="Shared")
    shard = [nc.dram_tensor(f"shard{k}", [NL, F], dt.bfloat16) for k in range(2)]
    tables = [x_table, table1, table2]

    with tile.TileContext(nc) as tc:
        with (
            tc.tile_pool(name="persist", bufs=1) as pp,
            tc.tile_pool(name="g0", bufs=6) as gp0,
            tc.tile_pool(name="g1", bufs=6) as gp1,
            tc.tile_pool(name="g2", bufs=6) as gp2,
            tc.tile_pool(name="g3", bufs=6) as gp3,
            tc.tile_pool(name="ohp", bufs=8) as ohp,
            tc.tile_pool(name="outp", bufs=3) as outp,
            tc.tile_pool(name="psA", bufs=4, space="PSUM") as psA,
            tc.tile_pool(name="psB", bufs=2, space="PSUM") as psB,
        ):
            gpools = [gp0, gp1, gp2, gp3]
            # persistent tiles
            drel_t = [pp.tile([128, B * CB], dt.float32, tag=f"drel{a}", name=f"drel{a}") for a in range(2)]
            attr_t = [pp.tile([128, B * CB], dt.float32, tag=f"attr{a}", name=f"attr{a}") for a in range(2)]
            for a in range(2):
                nc.sync.dma_start(drel_t[a][:], drel_in[a][:])
                nc.sync.dma_start(attr_t[a][:], attr_in[a][:])
            iota_t = pp.tile([128, 128], dt.bfloat16, tag="iota")
            nc.sync.dma_start(iota_t[:], iota_in[:])
            w_t = pp.tile([128, 9, 128], dt.bfloat16, tag="w")
            nc.sync.dma_start(w_t[:], w_in[:].rearrange("(w i) o -> i w o", i=128))
            bias_t = pp.tile([128, 3 * F], dt.float32, tag="bias")
            nc.sync.dma_start(bias_t[:], bias_in[:])
            xT = pp.tile([128, NL], dt.bfloat16, tag="xT")
            nc.sync.dma_start(xT[:], xT0_in[:])
            sT = [pp.tile([128, NL], dt.bfloat16, tag=f"sT{a}", name=f"sT{a}") for a in range(2)]
            # one shared resident idx buffer, reloaded per conv
            idx_sb = pp.tile([128, IDXW], dt.int16, tag="idxsb")

            for k in range(3):
                table = tables[k]
                if k > 0:
                    nc.sync.dma_start(xT[:], shard[k - 1][:], transpose=True)
                # sparse convs
                for a in range(2):
                    nc.sync.dma_start(idx_sb[:], idx_in[a][:])
                    for b in range(B):
                        gts = []
                        for q in range(R):
                            gt = gpools[q].tile([128, CPR, F], dt.bfloat16, tag=f"g{q}", name=f"gt{q}")
                            for sl in range(n_slabs):
                                s0 = sl * SLAB
                                ns = min(SLAB, CPR - s0)
                                col0 = (q * B + b) * CPR * 8 + s0 * 8
                                n_idx = ns * 128
                                nc.gpsimd.dma_gather(
                                    out_ap=gt[:, s0 : s0 + ns, :],
                                    in_ap=table[q * SR : (q + 1) * SR, :],
                                    idxs_ap=idx_sb[:, col0 : col0 + ns * 8],
                                    num_idxs=n_idx,
                                    num_idxs_reg=n_idx,
                                    elem_size=F,
                                )
                            gts.append(gt)
                        ps = psA.tile([128, 128], dt.float32, tag="psA", name="psa")
                        for j in range(CB):
                            q, s = divmod(j, CPR)
                            c = b * CB + j
                            oh = ohp.tile([128, 128], dt.bfloat16, tag="oh", name="oh")
                            nc.vector.tensor_scalar(
                                out=oh[:],
                                in0=iota_t[:],
                                scalar1=drel_t[a][:, c : c + 1],
                                scalar2=attr_t[a][:, c : c + 1],
                                op0=mybir.AluOpType.is_equal,
                                op1=mybir.AluOpType.mult,
                            )
                            nc.tensor.matmul(
                                ps[:],
                                gts[q][:, s, :],
                                oh[:],
                                start=(j == 0),
                                stop=(j == CB - 1),
                            )
                        nc.scalar.copy(sT[a][:, b * 128 : (b + 1) * 128], ps[:])
                # dense
                for b in range(B):
                    sl = slice(b * 128, (b + 1) * 128)
                    po = psB.tile([128, F], dt.float32, tag="psB", name="psb")
                    nc.tensor.matmul(po[:], sT[0][:, sl], w_t[:, k * 3 + 1, :], start=True, stop=False)
                    nc.tensor.matmul(po[:], sT[1][:, sl], w_t[:, k * 3 + 2, :], start=False, stop=False)
                    nc.tensor.matmul(po[:], xT[:, sl], w_t[:, k * 3 + 0, :], start=False, stop=True)
                    if k < 2:
                        ob = outp.tile([128, F], dt.bfloat16, tag="ob_bf", name="ob_bf")
                        nc.vector.tensor_tensor(
                            out=ob[:], in0=po[:], in1=bias_t[:, k * F : (k + 1) * F],
                            op=mybir.AluOpType.add,
                        )
                        nc.sync.dma_start(shard[k][sl, :], ob[:])
                    else:
                        ob = outp.tile([128, F], dt.float32, tag="ob_f32", name="ob_f32")
                        nc.vector.tensor_tensor(
                            out=ob[:], in0=po[:], in1=bias_t[:, k * F : (k + 1) * F],
                            op=mybir.AluOpType.add,
                        )
                        nc.sync.dma_start(out_p[sl, :], ob[:])
                if k < 2:
                    nc.gpsimd.collective_compute(
                        "AllGather",
                        mybir.AluOpType.bypass,
                        replica_groups=[list(range(NCORES))],
                        ins=[shard[k][:]],
                        outs=[tables[k + 1][:]],
                    )
    nc.finalize()
    return nc


def _run(x, edge_index, edge_attr, edge_index2, edge_attr2, weights, biases, NPAD,
         trace=False):
    """weights: [(W0,W1,W2)]*3 ; biases: [b_combined]*3 (already summed)."""
    NL = NPAD // NCORES
    n = x.shape[0]

    adjs = []
    maxc = 0
    for (src, dst), attr in (
        (edge_index, edge_attr),
        (edge_index2, edge_attr2),
    ):
        pc, mc = _prep_adjacency(
            np.asarray(src, np.int64), np.asarray(dst, np.int64), attr, NPAD
        )
        adjs.append(pc)
        maxc = max(maxc, mc)
    CPR = max(1, -(-maxc // 128))
    data = [_finalize_adjacency(pc, CPR, NPAD) for pc in adjs]

    xpad = np.zeros((NPAD, x.shape[1]), np.float32)
    xpad[:n] = x
    xtab = xpad.astype(BF16)

    wstack = np.concatenate(
        [np.asarray(w, np.float32) for trio in weights for w in trio], axis=0
    ).astype(BF16)  # [9*128, 128]
    bstack = np.concatenate(
        [np.tile(np.asarray(b, np.float32)[None, :], (128, 1)) for b in biases], axis=1
    ).astype(np.float32)  # [128, 3*128]
    iota = np.tile(np.arange(128, dtype=np.float32)[None, :], (128, 1)).astype(BF16)

    in_maps = []
    for r in range(NCORES):
        xT0 = np.ascontiguousarray(xtab[r * NL : (r + 1) * NL].T)
        in_maps.append(
            {
                "input0": xtab,
                "input1": xT0,
                "input2": data[0][0][r],
                "input3": data[1][0][r],
                "input4": data[0][1][r],
                "input5": data[1][1][r],
                "input6": data[0][2][r],
                "input7": data[1][2][r],
                "input8": wstack,
                "input9": bstack,
                "input10": iota,
            }
        )

    nc = _build_kernel(NPAD, CPR)
    res = run_bass_kernel_spmd(nc, in_maps, list(range(NCORES)), trace=trace)
    out = np.concatenate([res.results[r]["output0"] for r in range(NCORES)], axis=0)
    return out[:n], res


def kernel(**inputs):
    x = np.asarray(inputs["x"], np.float32)
    weights = []
    biases = []
    for blk in ("b1", "b2", "b3"):
        weights.append(
            (
                np.asarray(inputs[f"{blk}_ln_w"], np.float32),
                np.asarray(inputs[f"{blk}_c1_w"], np.float32),
                np.asarray(inputs[f"{blk}_c2_w"], np.float32),
            )
        )
        biases.append(
            np.asarray(inputs[f"{blk}_ln_b"], np.float32)
            + np.asarray(inputs[f"{blk}_c1_b"], np.float32)
            + np.asarray(inputs[f"{blk}_c2_b"], np.float32)
        )
    out, _ = _run(
        x,
        np.asarray(inputs["edge_index"]),
        np.asarray(inputs["edge_attr"], np.float32),
        np.asarray(inputs["edge_index2"]),
        np.asarray(inputs["edge_attr2"], np.float32),
        weights,
        biases,
        NPAD,
    )
    return out


# revision 5
# speedup vs baseline: 1.6829x; 1.6829x over previous
"""DiGCN inception-block (3 layers, 2 adjacencies) on 8 TRN2 NeuronCores.

Strategy (dst-sharded graph parallelism):
  - Nodes are partitioned across the 8 cores (12544 rows each, node space
    padded to 100352). Each core owns the output rows for its node shard.
  - Per layer: x_{k+1} = x_k @ W0 + (A1 @ x_k) @ W1 + (A2 @ x_k) @ W2 + b
    (using A @ (x W) == (A x) W, so the sparse ops run on raw x).
  - Sparse op A @ x: edges are sorted by destination block (128 dst rows)
    on the host, grouped into chunks of 128 edges. For each chunk the
    source rows are fetched with dma_gather (bf16, 256B rows) from the
    replicated x table in HBM, and a one-hot matrix
    onehot[e, d] = attr[e] * (dstrel[e] == d) is built on the vector
    engine with a single fused tensor_scalar(is_equal, mult). The
    TensorEngine then accumulates psum[feat, dst] += G_chunk.T @ onehot
    over all chunks of the block (PSUM accumulation = segment sum).
  - dma_gather indices are int16, so the x table is addressed in 4 ranges
    of 25088 rows; each (block, range) group is padded to a uniform chunk
    count so the SPMD program is identical on every core.
  - Dense part: per 128-node block, out_psum[node, feat] accumulates
    s1T/s2T/xT slices (feat-major lhsT) against the 128x128 weights.
  - Between layers the bf16 node shards are AllGathered into the next
    x table (internal Shared DRAM); layer 3 writes f32 shards directly.
"""

import sys

sys.path.insert(0, "/opt/trn_rl_repo")

import numpy as np
import ml_dtypes

from concourse import bass, mybir, bacc
import concourse.tile as tile
from concourse.bass_utils import run_bass_kernel_spmd

BF16 = ml_dtypes.bfloat16

NCORES = 8
F = 128  # feature dim (both in and out)
N = 100000  # real node count
NPAD = 100352  # 8 * 12544, multiple of 8*128
R = 4  # src ranges (int16 gather index limit)


def _prep_adjacency(src, dst, attr, NPAD):
    """Pack one adjacency into the uniform per-core grid.

    Returns (CPR-independent) per-core intermediate lists; final arrays are
    built once a global CPR is chosen.
    """
    NL = NPAD // NCORES
    B = NL // 128
    SR = NPAD // R
    per_core = []
    core = dst // NL
    for r in range(NCORES):
        m = core == r
        s = src[m].astype(np.int64)
        d = (dst[m] - r * NL).astype(np.int64)
        a = attr[m].astype(np.float32)
        b = d >> 7
        drel = (d & 127).astype(np.float32)
        q = s // SR
        srel = (s - q * SR).astype(np.int16)
        key = (b * R + q).astype(np.int64)
        order = np.argsort(key, kind="stable")
        key_s = key[order]
        counts = np.bincount(key_s, minlength=B * R)
        starts = np.concatenate([[0], np.cumsum(counts)[:-1]])
        pos = np.arange(len(key_s)) - starts[key_s]
        per_core.append((key_s, pos, srel[order], drel[order], a[order], counts))
    max_count = max(int(pc[5].max()) for pc in per_core) if len(src) else 0
    return per_core, max_count


def _finalize_adjacency(per_core, CPR, NPAD):
    NL = NPAD // NCORES
    B = NL // 128
    CB = R * CPR
    cap = CPR * 128
    idx_arrs, drel_arrs, attr_arrs = [], [], []
    for key_s, pos, srel, drel, a, counts in per_core:
        grid_src = np.zeros((B, R, cap), np.int16)
        grid_drel = np.zeros((B, R, cap), np.float32)
        grid_attr = np.zeros((B, R, cap), np.float32)
        bq_b = key_s // R
        bq_q = key_s % R
        grid_src[bq_b, bq_q, pos] = srel
        grid_drel[bq_b, bq_q, pos] = drel
        grid_attr[bq_b, bq_q, pos] = a
        # idx input: ranges-major, block-major inside: [R, B, cap] tokens.
        tokens = grid_src.transpose(1, 0, 2).reshape(-1)  # [R*B*cap]
        wrapped = np.tile(tokens.reshape(-1, 16).T, (8, 1))  # [128, R*B*cap/16]
        idx_arrs.append(np.ascontiguousarray(wrapped))
        # dstrel/attr: [128, B*CB] with col = b*CB + q*CPR + s, row = p
        dr = grid_drel.reshape(B, R, CPR, 128).transpose(3, 0, 1, 2).reshape(128, B * CB)
        at = grid_attr.reshape(B, R, CPR, 128).transpose(3, 0, 1, 2).reshape(128, B * CB)
        drel_arrs.append(np.ascontiguousarray(dr))
        attr_arrs.append(np.ascontiguousarray(at))
    return idx_arrs, drel_arrs, attr_arrs


def _build_kernel(NPAD, CPR):
    NL = NPAD // NCORES
    B = NL // 128
    CB = R * CPR
    SR = NPAD // R
    IDXW = R * B * CPR * 8  # idx free dim (int16 cols)
    # dma_gather is limited to 1024 indices per call; split each
    # (block, range) group into slabs of <=8 chunk-slots.
    SLAB = 8
    n_slabs = (CPR + SLAB - 1) // SLAB

    nc = bacc.Bacc("TRN2", target_bir_lowering=False, debug=False, num_devices=NCORES,
                   num_swdge_queues=4)
    dt = mybir.dt
    x_table = nc.declare_dram_parameter("input0", [NPAD, F], dt.bfloat16, isOutput=False)
    xT0_in = nc.declare_dram_parameter("input1", [128, NL], dt.bfloat16, isOutput=False)
    idx_in = [
        nc.declare_dram_parameter(f"input{2 + i}", [128, IDXW], dt.int16, isOutput=False)
        for i in range(2)
    ]
    drel_in = [
        nc.declare_dram_parameter(f"input{4 + i}", [128, B * CB], dt.float32, isOutput=False)
        for i in range(2)
    ]
    attr_in = [
        nc.declare_dram_parameter(f"input{6 + i}", [128, B * CB], dt.float32, isOutput=False)
        for i in range(2)
    ]
    w_in = nc.declare_dram_parameter("input8", [9 * 128, F], dt.bfloat16, isOutput=False)
    bias_in = nc.declare_dram_parameter("input9", [128, 3 * F], dt.float32, isOutput=False)
    iota_in = nc.declare_dram_parameter("input10", [128, 128], dt.bfloat16, isOutput=False)
    out_p = nc.declare_dram_parameter("output0", [NL, F], dt.float32, isOutput=True)

    table1 = nc.dram_tensor("table1", [NPAD, F], dt.bfloat16, addr_space="Shared")
    table2 = nc.dram_tensor("table2", [NPAD, F], dt.bfloat16, addr_space="Shared")
    shard = [nc.dram_tensor(f"shard{k}", [NL, F], dt.bfloat16) for k in range(2)]
    tables = [x_table, table1, table2]

    with tile.TileContext(nc) as tc:
        with (
            tc.tile_pool(name="persist", bufs=1) as pp,
            tc.tile_pool(name="g0", bufs=6) as gp0,
            tc.tile_pool(name="g1", bufs=6) as gp1,
            tc.tile_pool(name="g2", bufs=6) as gp2,
            tc.tile_pool(name="g3", bufs=6) as gp3,
            tc.tile_pool(name="ohp", bufs=8) as ohp,
            tc.tile_pool(name="outp", bufs=3) as outp,
            tc.tile_pool(name="psA", bufs=4, space="PSUM") as psA,
            tc.tile_pool(name="psB", bufs=2, space="PSUM") as psB,
        ):
            gpools = [gp0, gp1, gp2, gp3]
            # persistent tiles
            drel_t = [pp.tile([128, B * CB], dt.float32, tag=f"drel{a}", name=f"drel{a}") for a in range(2)]
            attr_t = [pp.tile([128, B * CB], dt.float32, tag=f"attr{a}", name=f"attr{a}") for a in range(2)]
            for a in range(2):
                nc.sync.dma_start(drel_t[a][:], drel_in[a][:])
                nc.sync.dma_start(attr_t[a][:], attr_in[a][:])
            iota_t = pp.tile([128, 128], dt.bfloat16, tag="iota")
            nc.sync.dma_start(iota_t[:], iota_in[:])
            w_t = pp.tile([128, 9, 128], dt.bfloat16, tag="w")
            nc.sync.dma_start(w_t[:], w_in[:].rearrange("(w i) o -> i w o", i=128))
            bias_t = pp.tile([128, 3 * F], dt.float32, tag="bias")
            nc.sync.dma_start(bias_t[:], bias_in[:])
            xT = pp.tile([128, NL], dt.bfloat16, tag="xT")
            nc.sync.dma_start(xT[:], xT0_in[:])
            sT = [pp.tile([128, NL], dt.bfloat16, tag=f"sT{a}", name=f"sT{a}") for a in range(2)]
            # one shared resident idx buffer, reloaded per conv
            idx_sb = pp.tile([128, IDXW], dt.int16, tag="idxsb")

            for k in range(3):
                table = tables[k]
                if k > 0:
                    nc.sync.dma_start(xT[:], shard[k - 1][:], transpose=True)
                # sparse convs
                for a in range(2):
                    nc.sync.dma_start(idx_sb[:], idx_in[a][:])
                    for b in range(B):
                        gts = []
                        for q in range(R):
                            gt = gpools[q].tile([128, CPR, F], dt.bfloat16, tag=f"g{q}", name=f"gt{q}")
                            for sl in range(n_slabs):
                                s0 = sl * SLAB
                                ns = min(SLAB, CPR - s0)
                                col0 = (q * B + b) * CPR * 8 + s0 * 8
                                n_idx = ns * 128
                                nc.gpsimd.dma_gather(
                                    out_ap=gt[:, s0 : s0 + ns, :],
                                    in_ap=table[q * SR : (q + 1) * SR, :],
                                    idxs_ap=idx_sb[:, col0 : col0 + ns * 8],
                                    num_idxs=n_idx,
                                    num_idxs_reg=n_idx,
                                    elem_size=F,
                                    queue_num=q,
                                )
                            gts.append(gt)
                        ps = psA.tile([128, 128], dt.float32, tag="psA", name="psa")
                        for j in range(CB):
                            q, s = divmod(j, CPR)
                            c = b * CB + j
                            oh = ohp.tile([128, 128], dt.bfloat16, tag="oh", name="oh")
                            nc.vector.tensor_scalar(
                                out=oh[:],
                                in0=iota_t[:],
                                scalar1=drel_t[a][:, c : c + 1],
                                scalar2=attr_t[a][:, c : c + 1],
                                op0=mybir.AluOpType.is_equal,
                                op1=mybir.AluOpType.mult,
                            )
                            nc.tensor.matmul(
                                ps[:],
                                gts[q][:, s, :],
                                oh[:],
                                start=(j == 0),
                                stop=(j == CB - 1),
                            )
                        nc.scalar.copy(sT[a][:, b * 128 : (b + 1) * 128], ps[:])
                # dense
                for b in range(B):
                    sl = slice(b * 128, (b + 1) * 128)
                    po = psB.tile([128, F], dt.float32, tag="psB", name="psb")
                    nc.tensor.matmul(po[:], sT[0][:, sl], w_t[:, k * 3 + 1, :], start=True, stop=False)
                    nc.tensor.matmul(po[:], sT[1][:, sl], w_t[:, k * 3 + 2, :], start=False, stop=False)
                    nc.tensor.matmul(po[:], xT[:, sl], w_t[:, k * 3 + 0, :], start=False, stop=True)
                    if k < 2:
                        ob = outp.tile([128, F], dt.bfloat16, tag="ob_bf", name="ob_bf")
                        nc.vector.tensor_tensor(
                            out=ob[:], in0=po[:], in1=bias_t[:, k * F : (k + 1) * F],
                            op=mybir.AluOpType.add,
                        )
                        nc.sync.dma_start(shard[k][sl, :], ob[:])
                    else:
                        ob = outp.tile([128, F], dt.float32, tag="ob_f32", name="ob_f32")
                        nc.vector.tensor_tensor(
                            out=ob[:], in0=po[:], in1=bias_t[:, k * F : (k + 1) * F],
                            op=mybir.AluOpType.add,
                        )
                        nc.sync.dma_start(out_p[sl, :], ob[:])
                if k < 2:
                    nc.gpsimd.collective_compute(
                        "AllGather",
                        mybir.AluOpType.bypass,
                        replica_groups=[list(range(NCORES))],
                        ins=[shard[k][:]],
                        outs=[tables[k + 1][:]],
                    )
    nc.finalize()
    return nc


def _run(x, edge_index, edge_attr, edge_index2, edge_attr2, weights, biases, NPAD,
         trace=False):
    """weights: [(W0,W1,W2)]*3 ; biases: [b_combined]*3 (already summed)."""
    NL = NPAD // NCORES
    n = x.shape[0]

    adjs = []
    maxc = 0
    for (src, dst), attr in (
        (edge_index, edge_attr),
        (edge_index2, edge_attr2),
    ):
        pc, mc = _prep_adjacency(
            np.asarray(src, np.int64), np.asarray(dst, np.int64), attr, NPAD
        )
        adjs.append(pc)
        maxc = max(maxc, mc)
    CPR = max(1, -(-maxc // 128))
    data = [_finalize_adjacency(pc, CPR, NPAD) for pc in adjs]

    xpad = np.zeros((NPAD, x.shape[1]), np.float32)
    xpad[:n] = x
    xtab = xpad.astype(BF16)

    wstack = np.concatenate(
        [np.asarray(w, np.float32) for trio in weights for w in trio], axis=0
    ).astype(BF16)  # [9*128, 128]
    bstack = np.concatenate(
        [np.tile(np.asarray(b, np.float32)[None, :], (128, 1)) for b in biases], axis=1
    ).astype(np.float32)  # [128, 3*128]
    iota = np.tile(np.arange(128, dtype=np.float32)[None, :], (128, 1)).astype(BF16)

    in_maps = []
    for r in range(NCORES):
        xT0 = np.ascontiguousarray(xtab[r * NL : (r + 1) * NL].T)
        in_maps.append(
            {
                "input0": xtab,
                "input1": xT0,
                "input2": data[0][0][r],
                "input3": data[1][0][r],
                "input4": data[0][1][r],
                "input5": data[1][1][r],
                "input6": data[0][2][r],
                "input7": data[1][2][r],
                "input8": wstack,
                "input9": bstack,
                "input10": iota,
            }
        )

    nc = _build_kernel(NPAD, CPR)
    res = run_bass_kernel_spmd(nc, in_maps, list(range(NCORES)), trace=trace)
    out = np.concatenate([res.results[r]["output0"] for r in range(NCORES)], axis=0)
    return out[:n], res


def kernel(**inputs):
    x = np.asarray(inputs["x"], np.float32)
    weights = []
    biases = []
    for blk in ("b1", "b2", "b3"):
        weights.append(
            (
                np.asarray(inputs[f"{blk}_ln_w"], np.float32),
                np.asarray(inputs[f"{blk}_c1_w"], np.float32),
                np.asarray(inputs[f"{blk}_c2_w"], np.float32),
            )
        )
        biases.append(
            np.asarray(inputs[f"{blk}_ln_b"], np.float32)
            + np.asarray(inputs[f"{blk}_c1_b"], np.float32)
            + np.asarray(inputs[f"{blk}_c2_b"], np.float32)
        )
    out, _ = _run(
        x,
        np.asarray(inputs["edge_index"]),
        np.asarray(inputs["edge_attr"], np.float32),
        np.asarray(inputs["edge_index2"]),
        np.asarray(inputs["edge_attr2"], np.float32),
        weights,
        biases,
        NPAD,
    )
    return out


# revision 6
# speedup vs baseline: 2.7456x; 1.6315x over previous
"""DiGCN inception-block (3 layers, 2 adjacencies) on 8 TRN2 NeuronCores.

Strategy (dst-sharded graph parallelism):
  - Nodes are partitioned across the 8 cores (12544 rows each, node space
    padded to 100352). Each core owns the output rows for its node shard.
  - Per layer: x_{k+1} = x_k @ W0 + (A1 @ x_k) @ W1 + (A2 @ x_k) @ W2 + b
    (using A @ (x W) == (A x) W, so the sparse ops run on raw x).
  - Sparse op A @ x: edges are sorted by destination block (128 dst rows)
    on the host, grouped into chunks of 128 edges. For each chunk the
    source rows are fetched with dma_gather (bf16, 256B rows) from the
    replicated x table in HBM, and a one-hot matrix
    onehot[e, d] = attr[e] * (dstrel[e] == d) is built on the vector
    engine with a single fused tensor_scalar(is_equal, mult). The
    TensorEngine then accumulates psum[feat, dst] += G_chunk.T @ onehot
    over all chunks of the block (PSUM accumulation = segment sum).
  - dma_gather indices are int16, so the x table is addressed in 4 ranges
    of 25088 rows; each (block, range) group is padded to a uniform chunk
    count so the SPMD program is identical on every core.
  - Dense part: per 128-node block, out_psum[node, feat] accumulates
    s1T/s2T/xT slices (feat-major lhsT) against the 128x128 weights.
  - Between layers the bf16 node shards are AllGathered into the next
    x table (internal Shared DRAM); layer 3 writes f32 shards directly.
"""

import sys

sys.path.insert(0, "/opt/trn_rl_repo")

import numpy as np
import ml_dtypes

from concourse import bass, mybir, bacc
import concourse.tile as tile
from concourse.bass_utils import run_bass_kernel_spmd

BF16 = ml_dtypes.bfloat16

NCORES = 8
F = 128  # feature dim (both in and out)
N = 100000  # real node count
NPAD = 100352  # 8 * 12544, multiple of 8*128
R = 4  # src ranges (int16 gather index limit)


def _prep_adjacency(src, dst, attr, NPAD):
    """Pack one adjacency into the uniform per-core grid.

    Returns (CPR-independent) per-core intermediate lists; final arrays are
    built once a global CPR is chosen.
    """
    NL = NPAD // NCORES
    B = NL // 128
    SR = NPAD // R
    per_core = []
    core = dst // NL
    for r in range(NCORES):
        m = core == r
        s = src[m].astype(np.int64)
        d = (dst[m] - r * NL).astype(np.int64)
        a = attr[m].astype(np.float32)
        b = d >> 7
        drel = (d & 127).astype(np.float32)
        q = s // SR
        srel = (s - q * SR).astype(np.int16)
        key = (b * R + q).astype(np.int64)
        order = np.argsort(key, kind="stable")
        key_s = key[order]
        counts = np.bincount(key_s, minlength=B * R)
        starts = np.concatenate([[0], np.cumsum(counts)[:-1]])
        pos = np.arange(len(key_s)) - starts[key_s]
        per_core.append((key_s, pos, srel[order], drel[order], a[order], counts))
    max_count = max(int(pc[5].max()) for pc in per_core) if len(src) else 0
    return per_core, max_count


def _finalize_adjacency(per_core, CPR, NPAD):
    NL = NPAD // NCORES
    B = NL // 128
    CB = R * CPR
    cap = CPR * 128
    idx_arrs, drel_arrs, attr_arrs = [], [], []
    for key_s, pos, srel, drel, a, counts in per_core:
        grid_src = np.zeros((B, R, cap), np.int16)
        grid_drel = np.zeros((B, R, cap), np.float32)
        grid_attr = np.zeros((B, R, cap), np.float32)
        bq_b = key_s // R
        bq_q = key_s % R
        grid_src[bq_b, bq_q, pos] = srel
        grid_drel[bq_b, bq_q, pos] = drel
        grid_attr[bq_b, bq_q, pos] = a
        # idx input: ranges-major, block-major inside: [R, B, cap] tokens.
        tokens = grid_src.transpose(1, 0, 2).reshape(-1)  # [R*B*cap]
        wrapped = np.tile(tokens.reshape(-1, 16).T, (8, 1))  # [128, R*B*cap/16]
        idx_arrs.append(np.ascontiguousarray(wrapped))
        # dstrel/attr: [128, B*CB] with col = b*CB + q*CPR + s, row = p
        dr = grid_drel.reshape(B, R, CPR, 128).transpose(3, 0, 1, 2).reshape(128, B * CB)
        at = grid_attr.reshape(B, R, CPR, 128).transpose(3, 0, 1, 2).reshape(128, B * CB)
        drel_arrs.append(np.ascontiguousarray(dr).astype(BF16))
        attr_arrs.append(np.ascontiguousarray(at).astype(BF16))
    return idx_arrs, drel_arrs, attr_arrs


def _build_kernel(NPAD, CPR):
    NL = NPAD // NCORES
    B = NL // 128
    CB = R * CPR
    SR = NPAD // R
    IDXW = R * B * CPR * 8  # idx free dim (int16 cols)
    # dma_gather is limited to 1024 indices per call; split each
    # (block, range) group into slabs of <=8 chunk-slots.
    SLAB = 8
    n_slabs = (CPR + SLAB - 1) // SLAB

    nc = bacc.Bacc("TRN2", target_bir_lowering=False, debug=False, num_devices=NCORES,
                   num_swdge_queues=4)
    dt = mybir.dt
    x_table = nc.declare_dram_parameter("input0", [NPAD, F], dt.bfloat16, isOutput=False)
    xT0_in = nc.declare_dram_parameter("input1", [128, NL], dt.bfloat16, isOutput=False)
    idx_in = [
        nc.declare_dram_parameter(f"input{2 + i}", [128, IDXW], dt.int16, isOutput=False)
        for i in range(2)
    ]
    drel_in = [
        nc.declare_dram_parameter(f"input{4 + i}", [128, B * CB], dt.bfloat16, isOutput=False)
        for i in range(2)
    ]
    attr_in = [
        nc.declare_dram_parameter(f"input{6 + i}", [128, B * CB], dt.bfloat16, isOutput=False)
        for i in range(2)
    ]
    w_in = nc.declare_dram_parameter("input8", [9 * 128, F], dt.bfloat16, isOutput=False)
    bias_in = nc.declare_dram_parameter("input9", [128, 3 * F], dt.float32, isOutput=False)
    iota_in = nc.declare_dram_parameter("input10", [128, 128], dt.bfloat16, isOutput=False)
    out_p = nc.declare_dram_parameter("output0", [NL, F], dt.float32, isOutput=True)

    table1 = nc.dram_tensor("table1", [NPAD, F], dt.bfloat16, addr_space="Shared")
    table2 = nc.dram_tensor("table2", [NPAD, F], dt.bfloat16, addr_space="Shared")
    shard = [nc.dram_tensor(f"shard{k}", [NL, F], dt.bfloat16) for k in range(2)]
    tables = [x_table, table1, table2]

    with tile.TileContext(nc) as tc:
        with (
            tc.tile_pool(name="persist", bufs=1) as pp,
            tc.tile_pool(name="g0", bufs=6) as gp0,
            tc.tile_pool(name="g1", bufs=6) as gp1,
            tc.tile_pool(name="g2", bufs=6) as gp2,
            tc.tile_pool(name="g3", bufs=6) as gp3,
            tc.tile_pool(name="ohp", bufs=4) as ohp,
            tc.tile_pool(name="outp", bufs=3) as outp,
            tc.tile_pool(name="psA", bufs=4, space="PSUM") as psA,
            tc.tile_pool(name="psB", bufs=2, space="PSUM") as psB,
        ):
            gpools = [gp0, gp1, gp2, gp3]
            # persistent tiles
            drel_t = [pp.tile([128, B * CB], dt.bfloat16, tag=f"drel{a}", name=f"drel{a}") for a in range(2)]
            attr_t = [pp.tile([128, B * CB], dt.bfloat16, tag=f"attr{a}", name=f"attr{a}") for a in range(2)]
            for a in range(2):
                nc.sync.dma_start(drel_t[a][:], drel_in[a][:])
                nc.sync.dma_start(attr_t[a][:], attr_in[a][:])
            iota_t = pp.tile([128, 128], dt.bfloat16, tag="iota")
            nc.sync.dma_start(iota_t[:], iota_in[:])
            w_t = pp.tile([128, 9, 128], dt.bfloat16, tag="w")
            nc.sync.dma_start(w_t[:], w_in[:].rearrange("(w i) o -> i w o", i=128))
            bias_t = pp.tile([128, 3 * F], dt.float32, tag="bias")
            nc.sync.dma_start(bias_t[:], bias_in[:])
            xT = pp.tile([128, NL], dt.bfloat16, tag="xT")
            nc.sync.dma_start(xT[:], xT0_in[:])
            sT = [pp.tile([128, NL], dt.bfloat16, tag=f"sT{a}", name=f"sT{a}") for a in range(2)]
            # one shared resident idx buffer, reloaded per conv
            idx_sb = pp.tile([128, IDXW], dt.int16, tag="idxsb")

            for k in range(3):
                table = tables[k]
                if k > 0:
                    nc.sync.dma_start(xT[:], shard[k - 1][:], transpose=True)
                # sparse convs
                for a in range(2):
                    nc.sync.dma_start(idx_sb[:], idx_in[a][:])
                    for b in range(B):
                        gts = []
                        for q in range(R):
                            gt = gpools[q].tile([128, CPR, F], dt.bfloat16, tag=f"g{q}", name=f"gt{q}")
                            for sl in range(n_slabs):
                                s0 = sl * SLAB
                                ns = min(SLAB, CPR - s0)
                                col0 = (q * B + b) * CPR * 8 + s0 * 8
                                n_idx = ns * 128
                                nc.gpsimd.dma_gather(
                                    out_ap=gt[:, s0 : s0 + ns, :],
                                    in_ap=table[q * SR : (q + 1) * SR, :],
                                    idxs_ap=idx_sb[:, col0 : col0 + ns * 8],
                                    num_idxs=n_idx,
                                    num_idxs_reg=n_idx,
                                    elem_size=F,
                                    queue_num=q,
                                )
                            gts.append(gt)
                        ps = psA.tile([128, 128], dt.float32, tag="psA", name="psa")
                        oh = ohp.tile([128, CB, 128], dt.bfloat16, tag="oh", name="oh")
                        iota_b = iota_t[:].unsqueeze(1).to_broadcast([128, CB, 128])
                        drel_b = (
                            drel_t[a][:, b * CB : (b + 1) * CB]
                            .unsqueeze(2)
                            .to_broadcast([128, CB, 128])
                        )
                        attr_b = (
                            attr_t[a][:, b * CB : (b + 1) * CB]
                            .unsqueeze(2)
                            .to_broadcast([128, CB, 128])
                        )
                        nc.vector.tensor_tensor(
                            out=oh[:], in0=iota_b, in1=drel_b, op=mybir.AluOpType.is_equal
                        )
                        nc.vector.tensor_tensor(
                            out=oh[:], in0=oh[:], in1=attr_b, op=mybir.AluOpType.mult
                        )
                        for j in range(CB):
                            q, s = divmod(j, CPR)
                            nc.tensor.matmul(
                                ps[:],
                                gts[q][:, s, :],
                                oh[:, j, :],
                                start=(j == 0),
                                stop=(j == CB - 1),
                            )
                        nc.scalar.copy(sT[a][:, b * 128 : (b + 1) * 128], ps[:])
                # dense
                for b in range(B):
                    sl = slice(b * 128, (b + 1) * 128)
                    po = psB.tile([128, F], dt.float32, tag="psB", name="psb")
                    nc.tensor.matmul(po[:], sT[0][:, sl], w_t[:, k * 3 + 1, :], start=True, stop=False)
                    nc.tensor.matmul(po[:], sT[1][:, sl], w_t[:, k * 3 + 2, :], start=False, stop=False)
                    nc.tensor.matmul(po[:], xT[:, sl], w_t[:, k * 3 + 0, :], start=False, stop=True)
                    if k < 2:
                        ob = outp.tile([128, F], dt.bfloat16, tag="ob_bf", name="ob_bf")
                        nc.vector.tensor_tensor(
                            out=ob[:], in0=po[:], in1=bias_t[:, k * F : (k + 1) * F],
                            op=mybir.AluOpType.add,
                        )
                        nc.sync.dma_start(shard[k][sl, :], ob[:])
                    else:
                        ob = outp.tile([128, F], dt.float32, tag="ob_f32", name="ob_f32")
                        nc.vector.tensor_tensor(
                            out=ob[:], in0=po[:], in1=bias_t[:, k * F : (k + 1) * F],
                            op=mybir.AluOpType.add,
                        )
                        nc.sync.dma_start(out_p[sl, :], ob[:])
                if k < 2:
                    nc.gpsimd.collective_compute(
                        "AllGather",
                        mybir.AluOpType.bypass,
                        replica_groups=[list(range(NCORES))],
                        ins=[shard[k][:]],
                        outs=[tables[k + 1][:]],
                    )
    nc.finalize()
    return nc


def _run(x, edge_index, edge_attr, edge_index2, edge_attr2, weights, biases, NPAD,
         trace=False):
    """weights: [(W0,W1,W2)]*3 ; biases: [b_combined]*3 (already summed)."""
    NL = NPAD // NCORES
    n = x.shape[0]

    adjs = []
    maxc = 0
    for (src, dst), attr in (
        (edge_index, edge_attr),
        (edge_index2, edge_attr2),
    ):
        pc, mc = _prep_adjacency(
            np.asarray(src, np.int64), np.asarray(dst, np.int64), attr, NPAD
        )
        adjs.append(pc)
        maxc = max(maxc, mc)
    CPR = max(1, -(-maxc // 128))
    data = [_finalize_adjacency(pc, CPR, NPAD) for pc in adjs]

    xpad = np.zeros((NPAD, x.shape[1]), np.float32)
    xpad[:n] = x
    xtab = xpad.astype(BF16)

    wstack = np.concatenate(
        [np.asarray(w, np.float32) for trio in weights for w in trio], axis=0
    ).astype(BF16)  # [9*128, 128]
    bstack = np.concatenate(
        [np.tile(np.asarray(b, np.float32)[None, :], (128, 1)) for b in biases], axis=1
    ).astype(np.float32)  # [128, 3*128]
    iota = np.tile(np.arange(128, dtype=np.float32)[None, :], (128, 1)).astype(BF16)

    in_maps = []
    for r in range(NCORES):
        xT0 = np.ascontiguousarray(xtab[r * NL : (r + 1) * NL].T)
        in_maps.append(
            {
                "input0": xtab,
                "input1": xT0,
                "input2": data[0][0][r],
                "input3": data[1][0][r],
                "input4": data[0][1][r],
                "input5": data[1][1][r],
                "input6": data[0][2][r],
                "input7": data[1][2][r],
                "input8": wstack,
                "input9": bstack,
                "input10": iota,
            }
        )

    nc = _build_kernel(NPAD, CPR)
    res = run_bass_kernel_spmd(nc, in_maps, list(range(NCORES)), trace=trace)
    out = np.concatenate([res.results[r]["output0"] for r in range(NCORES)], axis=0)
    return out[:n], res


def kernel(**inputs):
    x = np.asarray(inputs["x"], np.float32)
    weights = []
    biases = []
    for blk in ("b1", "b2", "b3"):
        weights.append(
            (
                np.asarray(inputs[f"{blk}_ln_w"], np.float32),
                np.asarray(inputs[f"{blk}_c1_w"], np.float32),
                np.asarray(inputs[f"{blk}_c2_w"], np.float32),
            )
        )
        biases.append(
            np.asarray(inputs[f"{blk}_ln_b"], np.float32)
            + np.asarray(inputs[f"{blk}_c1_b"], np.float32)
            + np.asarray(inputs[f"{blk}_c2_b"], np.float32)
        )
    out, _ = _run(
        x,
        np.asarray(inputs["edge_index"]),
        np.asarray(inputs["edge_attr"], np.float32),
        np.asarray(inputs["edge_index2"]),
        np.asarray(inputs["edge_attr2"], np.float32),
        weights,
        biases,
        NPAD,
    )
    return out
